# revision 9
# baseline (speedup 1.0000x reference)
"""Trainium2 Bass kernel for ConditionalTriangleAttention (v5).

Reference computation (B=2, N=40, HID=256, NH=8, CD=128, HD=32):
  cf = edge_features * sigmoid(cond@Wcg+bcg) + (cond@Wcp+bcp)     (per batch)
  Q/K/V = cf @ W_{q,k,v} + b                                       [B,N,N,NH,HD]
  scores = einsum('bijhd,bklhd->bijklh', Q, K)/sqrt(HD) + bias     (bias const over l)
  attn = softmax over l;  attended = einsum('bijklh,bklhd->bijhd', attn, V)
  out = (attended * sigmoid(attended@Wtg+btg)) @ Wo + bo

With edge_mask all-ones the additive bias (Wtb) is constant along the softmax
axis and cancels; likewise the K-side bias bk' contributes a per-(q,h)
constant to scores and cancels.  V's constant part contributes exactly
40*bv' (one per k-group) and is folded into the attT evacuation.
A numpy fallback handles any non-all-ones mask.

Sharding: 8 cores, each owns 400 query rows (b = core//4, i-row slice) and
computes all heads for them end-to-end -- no collectives.

v5 design (optimized for the CoreSim cost model):
 - edge_features are transposed + cast to fp8e4m3 on the host (layout prep
   only; all FLOPs stay on device).  kl is padded 1600->1680 so all 14
   chunks are full 120 rows (pad rows produce finite exps that are killed
   by zero indicator/V rows).
 - All projection matmuls (K/Q/V) and the score matmuls use fp8 DoubleRow
   (2 contraction tiles per instruction at 0.5 cycles/row).  Score matmuls
   zero-pad the second k-tile (hd=32 must stay 32-partition aligned).
 - Softmax: exp batched 3 chunks per ACT instruction (PSUM-limited),
   per-k-group sums via indicator matmul (bf16), reciprocal, DRAM bounce,
   broadcast-expansion DMA, then an in-place E *= Rbig on DVE (2x bf16).
 - Phase 3 gate/out projections run fp8-DoubleRow; sigmoid is computed as
   1/(1+exp(-x)) so the ACT engine only ever loads the Exp table.
"""

import os
import sys

for _p in ("/opt/trn_rl_repo", "/root/.axon_site/_ro/trn_rl_repo"):
    if os.path.isdir(_p) and _p not in sys.path:
        sys.path.insert(0, _p)

import numpy as np

B, N, HID, NH, CD = 2, 40, 256, 8, 128
HD = HID // NH            # 32
KL = N * N                # 1600
KLP = 1680                # padded kl (14 full chunks of 120)
NQ = KL // 4              # 400 query rows per core
NCORES = 8
ALPHA = 1.0 / np.sqrt(np.float32(HD))

CHUNK = 120
NCH = KLP // CHUNK        # 14
NG = 42                   # 40 real k-groups + 2 pad groups
BATCHES = ((0, 3), (3, 3), (6, 3), (9, 3), (12, 2))

_COMPILED = None


def _make_ind():
    # ind[p, ck, g] = 1 iff k-group of kl row (120*ck+p) == g, g = 3*ck + p//40
    ind = np.zeros((CHUNK, NCH, NG), np.float32)
    for ck in range(NCH):
        for p in range(CHUNK):
            ind[p, ck, 3 * ck + p // 40] = 1.0
    return ind


def _build_nc():
    import concourse.bass as bass
    import concourse.tile as tile
    from concourse import bacc, mybir

    FP = mybir.dt.float32
    BF = mybir.dt.bfloat16
    F8 = mybir.dt.float8e4
    AF = mybir.ActivationFunctionType
    DR = mybir.MatmulPerfMode.DoubleRow

    nc = bacc.Bacc(None, target_bir_lowering=False)

    ef8 = nc.dram_tensor("ef8", [128, 2 * KLP], F8, kind="ExternalInput")
    efq8 = nc.dram_tensor("efq8", [128, 2 * NQ], F8, kind="ExternalInput")
    zz = nc.dram_tensor("zz", [128, 2 * KLP], F8, kind="ExternalInput")
    cond = nc.dram_tensor("cond", [1, CD], FP, kind="ExternalInput")
    ind = nc.dram_tensor("ind", [CHUNK, NCH * NG], BF, kind="ExternalInput")
    # wcat rows (128-row units): Wq(2) Wk(2) Wv(2) Wtg(2) Wo(2) Wcp(1) Wcg(1)
    wcat = nc.dram_tensor("wcat", [12 * 128, HID], FP, kind="ExternalInput")
    # bcat rows: bq bk bv btg bo bcp bcg
    bcat = nc.dram_tensor("bcat", [1, 7 * HID], FP, kind="ExternalInput")
    out = nc.dram_tensor("out", [NQ, HID], FP, kind="ExternalOutput")
    r2 = nc.dram_tensor("r2", [8, NG * NQ], BF, kind="Internal")

    W_OFF = {"Wq": 0, "Wk": 2, "Wv": 4, "Wtg": 6, "Wo": 8, "Wcp": 10, "Wcg": 11}
    B_OFF = {"bq": 0, "bk": 1, "bv": 2, "btg": 3, "bo": 4, "bcp": 5, "bcg": 6}

    with tile.TileContext(nc) as tc:
        with tc.tile_pool(name="persist", bufs=1) as sb:
            ones11 = sb.tile([1, 1], FP, tag="ones11")
            nc.vector.memset(ones11, 1.0)
            onesb = sb.tile([1, 128], BF, tag="onesb")
            nc.vector.memset(onesb, 1.0)
            bob = sb.tile([1, HID], BF, tag="bob")

            ind_sb = sb.tile([CHUNK, NCH, NG], BF, tag="ind_sb")
            bcat_sb = sb.tile([1, 7, HID], FP, tag="bcat_sb")

            efT = sb.tile([128, 2, KLP], F8, tag="efT")
            efqT = sb.tile([128, 2, NQ], F8, tag="efqT")
            # KT/QT: [ch-in-half, qd, slot, kl]; slot 1 is zeros (DoubleRow pad)
            KT = sb.tile([128, 2, 2, KLP], F8, tag="KT")
            QT = sb.tile([128, 2, 2, NQ], F8, tag="QT")
            Vt = sb.tile([128, NCH, HID], BF, tag="Vt")
            attT = sb.tile([128, 2, NQ], BF, tag="attT")
            gatedT = sb.tile([128, 2, NQ], BF, tag="gatedT")
            # fp8 gated projection weights / bf16 phase-3 weights
            Wqp = sb.tile([128, 2, HID], F8, tag="Wqp")
            Wkp = sb.tile([128, 2, HID], F8, tag="Wkp")
            Wvp = sb.tile([128, 2, HID], F8, tag="Wvp")
            Wtgb = sb.tile([128, 2, HID], BF, tag="Wtgb")
            Wob = sb.tile([128, 2, HID], BF, tag="Wob")
            # per-partition columns
            gT = sb.tile([128, 2, 1], FP, tag="gT")
            pT = sb.tile([128, 2, 1], FP, tag="pT")
            bqT = sb.tile([128, 2, 1], FP, tag="bqT")
            bv40T = sb.tile([128, 2, 1], FP, tag="bv40T")
            btgn = sb.tile([128, 2, 1], FP, tag="btgn")
            osb = sb.tile([128, 4, HID], FP, tag="osb")

            # =============== phase 1: staging + cond + projections ===========
            with tc.tile_pool(name="wstage", bufs=1) as ws, \
                 tc.tile_pool(name="p1psum", bufs=2, space="PSUM") as pp, \
                 tc.tile_pool(name="kqpsum", bufs=2, space="PSUM") as kqp, \
                 tc.tile_pool(name="vpsum", bufs=2, space="PSUM") as vp:

                cond_sb = ws.tile([1, CD], FP, tag="cond_sb")
                nc.sync.dma_start(out=cond_sb[:], in_=cond[:])
                wst = ws.tile([128, 12, HID], FP, tag="wst")
                nc.sync.dma_start(
                    out=wst[:, :, :].opt(),
                    in_=bass.AP(tensor=wcat, offset=0,
                                ap=[[HID, 128], [128 * HID, 12], [1, HID]]))
                nc.gpsimd.dma_start(out=efT[:, :, :].opt(), in_=ef8[:, :])
                nc.gpsimd.dma_start(out=efqT[:, :, :].opt(), in_=efq8[:, :])
                # zero slots for the score DoubleRow pad
                nc.gpsimd.dma_start(out=KT[:, 0, 1, :], in_=zz[:, 0:KLP])
                nc.gpsimd.dma_start(out=KT[:, 1, 1, :], in_=zz[:, 0:KLP])
                nc.sync.dma_start(out=QT[:, 0, 1, :], in_=zz[:, 0:NQ])
                nc.sync.dma_start(out=QT[:, 1, 1, :], in_=zz[:, 0:NQ])
                nc.sync.dma_start(out=ind_sb[:, :, :].opt(), in_=ind[:, :])
                nc.sync.dma_start(out=bcat_sb[:, :, :].opt(), in_=bcat[:, :])

                def wslice(name, m, c0=0, cn=HID):
                    return wst[:, W_OFF[name] + m, c0:c0 + cn]

                def bslice(name, c0=0, cn=HID):
                    return bcat_sb[:, B_OFF[name], c0:c0 + cn]

                nc.vector.tensor_copy(bob[:, :], bslice("bo"))

                # ---- conditional gating columns ----
                ct_ps = pp.tile([128, 1], FP, tag="tiny")
                nc.tensor.matmul(out=ct_ps[:], lhsT=cond_sb[:], rhs=ones11[:],
                                 start=True, stop=True)
                condT = ws.tile([128, 1], FP, tag="condT")
                nc.vector.tensor_copy(condT[:], ct_ps[:])

                for m in range(2):
                    gp_ps = pp.tile([128, 1], FP, tag="tiny")
                    nc.tensor.matmul(out=gp_ps[:],
                                     lhsT=wslice("Wcg", 0, 128 * m, 128),
                                     rhs=condT[:], start=True, stop=False)
                    nc.tensor.matmul(out=gp_ps[:],
                                     lhsT=bslice("bcg", 128 * m, 128),
                                     rhs=ones11[:], start=False, stop=True)
                    # sigmoid via exp so ACT only ever loads the Exp table
                    nc.scalar.activation(out=gT[:, m, :], in_=gp_ps[:],
                                         func=AF.Exp, scale=-1.0)
                    nc.vector.tensor_scalar_add(gT[:, m, :], gT[:, m, :], 1.0)
                    nc.vector.reciprocal(gT[:, m, :], gT[:, m, :])

                    pp_ps = pp.tile([128, 1], FP, tag="tiny")
                    nc.tensor.matmul(out=pp_ps[:],
                                     lhsT=wslice("Wcp", 0, 128 * m, 128),
                                     rhs=condT[:], start=True, stop=False)
                    nc.tensor.matmul(out=pp_ps[:],
                                     lhsT=bslice("bcp", 128 * m, 128),
                                     rhs=ones11[:], start=False, stop=True)
                    nc.vector.tensor_copy(pT[:, m, :], pp_ps[:])

                # gated weights W' = diag(g) W  (fp8)
                for (wn, dst) in (("Wq", Wqp), ("Wk", Wkp), ("Wv", Wvp)):
                    for c in range(2):
                        nc.vector.tensor_scalar_mul(dst[:, c, :],
                                                    wslice(wn, c), gT[:, c, :])
                for (wn, dst) in (("Wtg", Wtgb), ("Wo", Wob)):
                    for c in range(2):
                        nc.vector.tensor_copy(dst[:, c, :], wslice(wn, c))

                # bias columns: bq' = (p @ Wq + bq)^T ; bv' * 40 ; -btg^T
                for m in range(2):
                    bps = pp.tile([128, 1], FP, tag="tiny")
                    for c in range(2):
                        nc.tensor.matmul(out=bps[:],
                                         lhsT=wslice("Wq", c, 128 * m, 128),
                                         rhs=pT[:, c, :], start=(c == 0),
                                         stop=False)
                    nc.tensor.matmul(out=bps[:],
                                     lhsT=bslice("bq", 128 * m, 128),
                                     rhs=ones11[:], start=False, stop=True)
                    nc.vector.tensor_copy(bqT[:, m, :], bps[:])

                    vps_c = pp.tile([128, 1], FP, tag="tiny")
                    for c in range(2):
                        nc.tensor.matmul(out=vps_c[:],
                                         lhsT=wslice("Wv", c, 128 * m, 128),
                                         rhs=pT[:, c, :], start=(c == 0),
                                         stop=False)
                    nc.tensor.matmul(out=vps_c[:],
                                     lhsT=bslice("bv", 128 * m, 128),
                                     rhs=ones11[:], start=False, stop=True)
                    nc.vector.tensor_scalar_mul(bv40T[:, m, :], vps_c[:], 40.0)

                    tps = pp.tile([128, 1], FP, tag="tiny")
                    nc.tensor.matmul(out=tps[:],
                                     lhsT=bslice("btg", 128 * m, 128),
                                     rhs=ones11[:], start=True, stop=True)
                    nc.vector.tensor_scalar_mul(btgn[:, m, :], tps[:], -1.0)

                # ---- K^T blocks (DoubleRow over the 2x128 contraction) ----
                KBLK = ((0, 512), (512, 512), (1024, 512), (1536, KLP - 1536))

                def k_block(m, o, w):
                    kps = kqp.tile([128, 512], FP, tag="kq")
                    nc.tensor.matmul(out=kps[:, 0:w],
                                     lhsT=Wkp[:, :, 128 * m:128 * (m + 1)],
                                     rhs=efT[:, :, o:o + w],
                                     start=True, stop=True, perf_mode=DR)
                    # bk' cancels in the l-softmax: pure copy evacuation
                    nc.vector.tensor_copy(KT[:, m, 0, o:o + w], kps[:, 0:w])

                def q_block(m):
                    qps = kqp.tile([128, 512], FP, tag="kq")
                    nc.tensor.matmul(out=qps[:, 0:NQ],
                                     lhsT=Wqp[:, :, 128 * m:128 * (m + 1)],
                                     rhs=efqT[:, :, :],
                                     start=True, stop=True, perf_mode=DR)
                    nc.vector.tensor_scalar_add(QT[:, m, 0, :], qps[:, 0:NQ],
                                                bqT[:, m, :])

                for (o, w) in KBLK:
                    k_block(0, o, w)
                q_block(0)
                for (o, w) in KBLK:
                    k_block(1, o, w)
                q_block(1)

                # ---- V^T chunks (DoubleRow) ----
                for ck in range(NCH):
                    vps = vp.tile([128, HID], FP, tag="vps", name=f"vps{ck}")
                    nc.tensor.matmul(out=vps[0:CHUNK, :],
                                     lhsT=efT[:, :, CHUNK * ck:CHUNK * (ck + 1)],
                                     rhs=Wvp[:, :, :],
                                     start=True, stop=True, perf_mode=DR)
                    nc.vector.tensor_copy(Vt[0:CHUNK, ck, :], vps[0:CHUNK, :])

            # =============== phase 2: attention, 4 software-pipelined units ==
            with tc.tile_pool(name="stp", bufs=2, space="PSUM") as stp_pool, \
                 tc.tile_pool(name="sums", bufs=1, space="PSUM") as sum_pool, \
                 tc.tile_pool(name="ups", bufs=1, space="PSUM") as ups_pool, \
                 tc.tile_pool(name="Epool", bufs=3) as E_pool, \
                 tc.tile_pool(name="Rpool", bufs=3) as R_pool, \
                 tc.tile_pool(name="rqpool", bufs=3) as rq_pool:

                units = [(qd, pair) for qd in range(2) for pair in range(2)]
                Etiles, Rtiles, upst = {}, {}, {}

                def emit_A(u):
                    qd, pair = units[u]
                    E = E_pool.tile([128, 2, NCH, NQ], BF, tag="E", name=f"E{u}")
                    Rb = R_pool.tile([128, 2, NCH, NQ], BF, tag="R", name=f"R{u}")
                    Etiles[u], Rtiles[u] = E, Rb
                    sums = sum_pool.tile([128, NQ], FP, tag="sums",
                                         name=f"sums{u}")
                    for t in range(2):
                        hh = 2 * pair + t
                        for (ck0, ckn) in BATCHES:
                            # 512-wide chunk slots keep each matmul output
                            # 2KB-aligned (PSUM zero-region granularity)
                            stp = stp_pool.tile([128, 3, 512], FP, tag="stp",
                                                name=f"stp{u}_{t}_{ck0}")
                            for k in range(ckn):
                                ck = ck0 + k
                                nc.tensor.matmul(
                                    out=stp[0:CHUNK, k, 0:NQ],
                                    lhsT=KT[32 * hh:32 * hh + 32, qd, :,
                                            CHUNK * ck:CHUNK * (ck + 1)],
                                    rhs=QT[32 * hh:32 * hh + 32, qd, :, :],
                                    start=True, stop=True, perf_mode=DR,
                                    tile_position=(32 * hh, 0))
                            nc.scalar.activation(
                                out=E[0:CHUNK, t, ck0:ck0 + ckn, :],
                                in_=stp[0:CHUNK, 0:ckn, 0:NQ],
                                func=AF.Exp, scale=float(ALPHA))
                            for k in range(ckn):
                                ck = ck0 + k
                                nc.tensor.matmul(
                                    out=sums[64 * t:64 * t + NG, :],
                                    lhsT=ind_sb[0:CHUNK, ck, :],
                                    rhs=E[0:CHUNK, t, ck, :],
                                    start=(ck == 0), stop=(ck == NCH - 1),
                                    skip_group_check=True)
                        # normalizers: reciprocal -> DRAM bounce -> expansion
                        slot = u * 2 + t
                        rq = rq_pool.tile([128, NQ], BF, tag="rq",
                                          name=f"rq{u}_{t}")
                        with nc.allow_low_precision(reason="softmax recip"):
                            nc.vector.reciprocal(rq[0:NG, :],
                                                 sums[64 * t:64 * t + NG, :])
                        nc.sync.dma_start(
                            out=bass.AP(tensor=r2, offset=slot * NG * NQ,
                                        ap=[[NQ, NG], [1, NQ]]),
                            in_=rq[0:NG, :])
                        for j in range(3):
                            src = bass.AP(tensor=r2,
                                          offset=slot * NG * NQ + j * NQ,
                                          ap=[[0, 40], [3 * NQ, NCH], [1, NQ]])
                            eng = (nc.sync, nc.gpsimd, nc.sync)[j]
                            eng.dma_start(
                                out=Rb[40 * j:40 * j + 40, t, :, :].opt(),
                                in_=src)

                def emit_B(u):
                    qd, pair = units[u]
                    E, Rb = Etiles.pop(u), Rtiles.pop(u)
                    if pair == 0:
                        upst[qd] = ups_pool.tile([128, NQ], FP, tag="ups",
                                                 name=f"ups{qd}")
                    ups = upst[qd]
                    for t in range(2):
                        nc.vector.tensor_mul(E[0:CHUNK, t, :, :],
                                             E[0:CHUNK, t, :, :],
                                             Rb[0:CHUNK, t, :, :])
                    for ck in range(NCH):
                        for t in range(2):
                            hh = 2 * pair + t
                            nc.tensor.matmul(
                                out=ups[32 * hh:32 * hh + 32, :],
                                lhsT=Vt[0:CHUNK, ck,
                                        128 * qd + 32 * hh:128 * qd + 32 * hh + 32],
                                rhs=E[0:CHUNK, t, ck, :],
                                start=(ck == 0), stop=(ck == NCH - 1),
                                skip_group_check=True,
                                tile_position=(0, 32 * hh))
                    if pair == 1:
                        with nc.allow_low_precision(reason="attT fp8"):
                            nc.vector.tensor_scalar_add(attT[:, qd, :], ups[:],
                                                        bv40T[:, qd, :])
                        del upst[qd]

                emit_A(0)
                emit_A(1)
                emit_B(0)
                emit_A(2)
                emit_B(1)
                emit_A(3)
                emit_B(2)
                emit_B(3)

            # ======== phase 3: gate + output projection (fp8 DoubleRow) =====
            with tc.tile_pool(name="p3psum", bufs=6, space="PSUM") as p3p, \
                 tc.tile_pool(name="p3sb", bufs=1) as p3s:
                gex = p3s.tile([128, 2, NQ], FP, tag="gex")
                gateT = p3s.tile([128, 2, NQ], BF, tag="gateT")
                for m in range(2):
                    gps = p3p.tile([128, NQ], FP, tag="p3", name=f"gps{m}")
                    for c in range(2):
                        nc.tensor.matmul(out=gps[:],
                                         lhsT=Wtgb[:, c, 128 * m:128 * (m + 1)],
                                         rhs=attT[:, c, :],
                                         start=(c == 0), stop=(c == 1))
                    # sigmoid(x+btg) = 1/(1+exp(-x-btg))
                    nc.scalar.activation(out=gex[:, m, :], in_=gps[:],
                                         func=AF.Exp, scale=-1.0,
                                         bias=btgn[:, m, :])
                    nc.vector.tensor_scalar_add(gex[:, m, :], gex[:, m, :], 1.0)
                    with nc.allow_low_precision(reason="gate recip"):
                        nc.vector.reciprocal(gateT[:, m, :], gex[:, m, :])
                    with nc.allow_low_precision(reason="gated fp8"):
                        nc.vector.tensor_mul(gatedT[:, m, :], attT[:, m, :],
                                             gateT[:, m, :])
                for qc in range(4):
                    o0 = qc * 100
                    ops = p3p.tile([128, HID], FP, tag="p3", name=f"ops{qc}")
                    for c in range(2):
                        nc.tensor.matmul(out=ops[0:100, :],
                                         lhsT=gatedT[:, c, o0:o0 + 100],
                                         rhs=Wob[:, c, :],
                                         start=(c == 0), stop=False)
                    nc.tensor.matmul(out=ops[0:100, :],
                                     lhsT=onesb[:, 0:100],
                                     rhs=bob[:, :], start=False, stop=True)
                    nc.vector.tensor_copy(osb[0:100, qc, :], ops[0:100, :])
                nc.sync.dma_start(
                    out=bass.AP(tensor=out, offset=0,
                                ap=[[HID, 100], [100 * HID, 4], [1, HID]]),
                    in_=osb[0:100, :, :].opt())

    nc.finalize()
    return nc


def _get_compiled():
    global _COMPILED
    if _COMPILED is None:
        _COMPILED = _build_nc()
    return _COMPILED


def _numpy_reference(edge_features, edge_mask, condition, Wq, bq, Wk, bk, Wv, bv,
                     Wcp, bcp, Wcg, bcg, Wtb, btb, Wtg, btg, Wo, bo):
    def sig(x):
        return 1.0 / (1.0 + np.exp(-x))
    cond_proj = condition @ Wcp + bcp
    cond_gate = sig(condition @ Wcg + bcg)
    cf = edge_features * cond_gate[:, None, None, :] + cond_proj[:, None, None, :]
    Q = (cf @ Wq + bq).reshape(B, N, N, NH, HD)
    K = (cf @ Wk + bk).reshape(B, N, N, NH, HD)
    V = (cf @ Wv + bv).reshape(B, N, N, NH, HD)
    scores = np.einsum('bijhd,bklhd->bijklh', Q, K) / np.sqrt(HD).astype(np.float32)
    bias_in = np.concatenate(
        [cf, np.broadcast_to(condition[:, None, None, :], (B, N, N, CD))], axis=-1)
    bias = bias_in @ Wtb + btb
    scores = scores + bias[:, :, :, None, None, :]
    m = edge_mask[:, None, None, :, :, None] & edge_mask[:, :, :, None, None, None]
    scores = np.where(m, scores, -np.inf)
    mx = np.max(scores, axis=4, keepdims=True)
    mx = np.where(np.isfinite(mx), mx, 0.0)
    e = np.exp(scores - mx)
    attn = e / np.maximum(np.sum(e, axis=4, keepdims=True), 1e-30)
    attended = np.einsum('bijklh,bklhd->bijhd', attn, V).reshape(B, N, N, HID)
    gate = sig(attended @ Wtg + btg)
    return ((attended * gate) @ Wo + bo).astype(np.float32)


def _make_in_maps(ins):
    import ml_dtypes
    F8NP = ml_dtypes.float8_e4m3

    ef_full = np.ascontiguousarray(ins["edge_features"].astype(np.float32)
                                   .reshape(B, KL, HID))
    condition = ins["condition"].astype(np.float32)

    f32 = lambda k: ins[k].astype(np.float32)
    wcat = np.concatenate([f32("Wq"), f32("Wk"), f32("Wv"), f32("Wtg"),
                           f32("Wo"), f32("Wcp"), f32("Wcg")], axis=0)
    bcat = np.concatenate([f32(k).reshape(-1) for k in
                           ("bq", "bk", "bv", "btg", "bo", "bcp", "bcg")]
                          ).reshape(1, -1)
    shared = {
        "wcat": np.ascontiguousarray(wcat),
        "bcat": np.ascontiguousarray(bcat),
        "ind": _make_ind().reshape(CHUNK, NCH * NG).astype(ml_dtypes.bfloat16),
        "zz": np.zeros((128, 2 * KLP), F8NP),
    }
    in_maps = []
    for c in range(NCORES):
        b, s = c // 4, c % 4
        m = dict(shared)
        # ef8[i, c, kl] = ef[kl, 128c+i], kl padded 1600->1680 with zeros
        efT = np.zeros((128, 2, KLP), np.float32)
        src = ef_full[b].T.reshape(2, 128, KL)            # [c, i, kl]
        efT[:, :, 0:KL] = src.transpose(1, 0, 2)
        m["ef8"] = efT.astype(F8NP).reshape(128, 2 * KLP)
        efq = ef_full[b, s * NQ:(s + 1) * NQ].T.reshape(2, 128, NQ)
        m["efq8"] = np.ascontiguousarray(
            efq.transpose(1, 0, 2)).astype(F8NP).reshape(128, 2 * NQ)
        m["cond"] = np.ascontiguousarray(condition[b:b + 1])
        in_maps.append(m)
    return in_maps


def kernel(**inputs):
    ins = {k: np.asarray(v) for k, v in inputs.items()}
    edge_mask = ins["edge_mask"]
    if not bool(edge_mask.all()):
        return _numpy_reference(
            ins["edge_features"].astype(np.float32), edge_mask.astype(bool),
            ins["condition"].astype(np.float32),
            *[ins[k].astype(np.float32) for k in
              ("Wq", "bq", "Wk", "bk", "Wv", "bv", "Wcp", "bcp", "Wcg", "bcg",
               "Wtb", "btb", "Wtg", "btg", "Wo", "bo")])

    in_maps = _make_in_maps(ins)
    from concourse.bass_utils import run_bass_kernel_spmd
    nc = _get_compiled()
    res = run_bass_kernel_spmd(nc, in_maps, core_ids=list(range(NCORES)))
    outs = [r["out"] for r in res.results]
    full = np.empty((B, KL, HID), np.float32)
    for c in range(NCORES):
        b, s = c // 4, c % 4
        full[b, s * NQ:(s + 1) * NQ] = outs[c]
    return full.reshape(B, N, N, HID)


if __name__ == "__main__":
    nc = _build_nc()
    print("built ok")
